# revision 3
# baseline (speedup 1.0000x reference)
"""Trainium2 Bass kernel for causal multi-head attention with RoPE.

Problem: B=2, S=2048, D=1024, H=16 heads, L=64 head dim, causal, interleaved
RoPE, fp32 reference.

Sharding (8 cores): data-parallel over batch (2 groups of 4 cores) x tensor
parallel over heads (4 heads per core).

Fully fused pipeline per core (single TileContext, no phase barriers):
  for each 512-seq chunk ch:
    attention q-block qb=ch (flash-style, transposed scores [k, q], exp on
    ScalarE, PV with a ones column for softmax denominators) runs ACT-bound;
    the PE-heavy QKV projection + RoPE for chunk ch+1 is interleaved into the
    kt loop so TensorE stays busy under the exp stream;
    normalization multiplies PSUM directly by broadcast reciprocals (fused
    cast), out-projection for qb=ch reuses the PV PSUM banks, and the
    4-core ReduceScatter for qb=ch is issued immediately so collectives
    overlap later q-blocks instead of forming a serial tail.

PSUM budget (8 banks): scores double-buffer psc0/psc1 (2+2 banks; projection
accumulators share these tags with sub-1.2us tenures so the exp stream never
starves) + one 4-bank "acc" tag rotating pvs(qb) -> outproj(qb).

Matmuls run in bf16 (fp32 PSUM accumulation).  The RoPE pair dimension is
host-permuted into separated halves (x0 cols then x1 cols) so on-chip RoPE is
6 dense tensor ops per tensor; the permutation is applied identically to Q and
K so dot products (scores) are unchanged.
"""

import sys

import numpy as np

for _p in ("/opt/trn_rl_repo",):
    if _p not in sys.path:
        sys.path.insert(0, _p)

import ml_dtypes

import concourse.bass as bass  # noqa: F401  (registers types)
import concourse.mybir as mybir
import concourse.tile as tile
from concourse import bacc
from concourse import bass_utils

BF16 = mybir.dt.bfloat16
F32 = mybir.dt.float32
NPBF16 = ml_dtypes.bfloat16
AF = mybir.ActivationFunctionType
ALU = mybir.AluOpType

B, S, D = 2, 2048, 1024
H, L = 16, 64
HPC = 4  # heads per core
N_CORES = 8
QB = 512  # query block (columns of transposed scores)
NQB = S // QB  # 4
NKT = S // 128  # 16 key tiles
ROPE_BASE = 10000.0
REPLICA_GROUPS = [[0, 1, 2, 3], [4, 5, 6, 7]]


def build_program():
    nc = bacc.Bacc(
        "TRN2", target_bir_lowering=False, debug=False, num_devices=N_CORES
    )

    # ---- I/O ----
    xt_d = nc.dram_tensor("xt", [D, S], BF16, kind="ExternalInput")
    wq0_d = nc.dram_tensor("wq0", [D, 128], BF16, kind="ExternalInput")
    wq1_d = nc.dram_tensor("wq1", [D, 128], BF16, kind="ExternalInput")
    wk0_d = nc.dram_tensor("wk0", [D, 128], BF16, kind="ExternalInput")
    wk1_d = nc.dram_tensor("wk1", [D, 128], BF16, kind="ExternalInput")
    wv_d = nc.dram_tensor("wv", [D, 256], BF16, kind="ExternalInput")
    wo_d = nc.dram_tensor("wo", [256, D], BF16, kind="ExternalInput")
    bq0_d = nc.dram_tensor("bq0", [128, 1], F32, kind="ExternalInput")
    bq1_d = nc.dram_tensor("bq1", [128, 1], F32, kind="ExternalInput")
    bk0_d = nc.dram_tensor("bk0", [128, 1], F32, kind="ExternalInput")
    bk1_d = nc.dram_tensor("bk1", [128, 1], F32, kind="ExternalInput")
    bvr_d = nc.dram_tensor("bvr", [1, 256], BF16, kind="ExternalInput")
    cos_d = nc.dram_tensor("cos4", [128, S], BF16, kind="ExternalInput")
    sin_d = nc.dram_tensor("sin4", [128, S], BF16, kind="ExternalInput")
    tri_d = nc.dram_tensor("tri", [128, 128], BF16, kind="ExternalInput")
    out_d = nc.dram_tensor("out", [4 * 128, D], BF16, kind="ExternalOutput")

    partial_d = nc.dram_tensor("partial", [S, D], BF16, kind="Internal")
    recip_d = nc.dram_tensor("recipd", [4 * HPC, 512], BF16, kind="Internal")
    rs_d = [
        nc.dram_tensor(f"rs{qb}", [128, D], BF16, kind="Internal")
        for qb in range(NQB)
    ]

    with tile.TileContext(nc) as tc:
        with (
            tc.tile_pool(name="const", bufs=1) as cpool,
            tc.tile_pool(name="xp", bufs=1) as xpool,
            tc.tile_pool(name="qk", bufs=1) as qkpool,
            tc.tile_pool(name="rtmp", bufs=1) as rtmp,
            tc.tile_pool(name="ptp", bufs=3) as ptpool,
            tc.tile_pool(name="att", bufs=1) as attpool,
            tc.tile_pool(name="bc", bufs=2) as bcpool,
            tc.tile_pool(name="osb", bufs=3) as opool,
            tc.tile_pool(name="ps", bufs=1, space="PSUM") as pspool,
        ):
            # ---- constants / weights ----
            def load_w(dram, cols):
                t = cpool.tile([128, 8, cols], BF16, tag=f"w_{dram.name}")
                nc.sync.dma_start(t[:], dram.ap().rearrange("(o p) m -> p o m", p=128))
                return t

            wq0_sb = load_w(wq0_d, 128)
            wq1_sb = load_w(wq1_d, 128)
            wk0_sb = load_w(wk0_d, 128)
            wk1_sb = load_w(wk1_d, 128)
            wv_sb = load_w(wv_d, 256)
            wo_sb = cpool.tile([128, 2, D], BF16)
            nc.sync.dma_start(wo_sb[:], wo_d.ap().rearrange("(o p) m -> p o m", p=128))

            def load_c(dram, shape, dt, tag):
                t = cpool.tile(shape, dt, tag=tag)
                nc.sync.dma_start(t[:], dram.ap())
                return t

            bq0_sb = load_c(bq0_d, [128, 1], F32, "bq0")
            bq1_sb = load_c(bq1_d, [128, 1], F32, "bq1")
            bk0_sb = load_c(bk0_d, [128, 1], F32, "bk0")
            bk1_sb = load_c(bk1_d, [128, 1], F32, "bk1")
            bvr_sb = load_c(bvr_d, [1, 256], BF16, "bvr")
            cos_sb = load_c(cos_d, [128, S], BF16, "cos4")
            sin_sb = load_c(sin_d, [128, S], BF16, "sin4")
            tri_sb = load_c(tri_d, [128, 128], BF16, "tri")

            ones_row = cpool.tile([1, 128], BF16, tag="ones_row")
            nc.vector.memset(ones_row[:], 1.0)

            # ---- persistent SBUF state ----
            xt_sb = xpool.tile([128, 8, S], BF16)
            xt_r = xt_d.ap().rearrange("(o p) s -> p o s", p=128)

            q0_sb = qkpool.tile([128, S], BF16, tag="q0")
            q1_sb = qkpool.tile([128, S], BF16, tag="q1")
            k0_sb = qkpool.tile([128, S], BF16, tag="k0")
            k1_sb = qkpool.tile([128, S], BF16, tag="k1")
            qm = [
                qkpool.tile([128, S], BF16, tag=f"qm{w}", name=f"qm{w}")
                for w in range(2)
            ]
            km = [
                qkpool.tile([128, S], BF16, tag=f"km{w}", name=f"km{w}")
                for w in range(2)
            ]
            # V with a ones column appended per head (65 cols/head): the PV
            # matmul then emits the softmax denominator in output row 64.
            v_sb = qkpool.tile([128, NKT, HPC * 65], BF16, tag="v")
            nc.vector.memset(
                v_sb[:].rearrange("p t (h c) -> p t h c", c=65)[:, :, :, 64:65], 1.0
            )

            atth_sb = attpool.tile([64, 2, QB], BF16, tag="atth")  # odd-head stage
            attp_sb = attpool.tile([128, 2, S], BF16, tag="attp")
            sums_sb = attpool.tile([128, 64], F32, tag="sums")
            recip_sb = attpool.tile([128, 64], F32, tag="recip")
            stg_sb = attpool.tile([1, HPC, QB], F32, tag="stg")
            rb_sb = attpool.tile([128, 64], BF16, tag="rb")

            tri_b2 = tri_sb[:, None, :].to_broadcast((128, 2, 128))

            def dma_xt(ch):
                for dt_ in range(8):
                    nc.sync.dma_start(
                        xt_sb[:, dt_, ch * 512 : (ch + 1) * 512],
                        xt_r[:, dt_, ch * 512 : (ch + 1) * 512],
                    )

            # ---- projection pieces for one 512-seq chunk ----
            # Each piece holds a psc-tag PSUM slot for <1.2us so the exp
            # stream (which alternates psc0/psc1) never starves.
            _ptag = [0]

            def ptag():
                _ptag[0] ^= 1
                return f"psc{_ptag[0]}"

            def make_pieces(ch):
                pieces = []

                def qk_piece(dst, w_sb, b_sb, sti, nm):
                    def f():
                        c0 = ch * 512 + sti * 256
                        ps = pspool.tile(
                            [128, 256], F32, tag=ptag(), name=f"pp_{nm}_{ch}_{sti}"
                        )
                        for dt_ in range(8):
                            nc.tensor.matmul(
                                ps[:],
                                w_sb[:, dt_, :],
                                xt_sb[:, dt_, c0 : c0 + 256],
                                start=(dt_ == 0),
                                stop=(dt_ == 7),
                            )
                        nc.vector.tensor_scalar(
                            dst[:, c0 : c0 + 256], ps[:], b_sb[:, 0:1], None, ALU.add
                        )

                    return f

                def rope_piece(x0, x1):
                    def f():
                        sl = slice(ch * 512, (ch + 1) * 512)
                        m1 = rtmp.tile([128, 512], BF16, tag="m1")
                        m2 = rtmp.tile([128, 512], BF16, tag="m2")
                        m3 = rtmp.tile([128, 512], BF16, tag="m3")
                        m4 = rtmp.tile([128, 512], BF16, tag="m4")
                        nc.vector.tensor_tensor(m1[:], x0[:, sl], cos_sb[:, sl], ALU.mult)
                        nc.vector.tensor_tensor(m2[:], x1[:, sl], sin_sb[:, sl], ALU.mult)
                        nc.vector.tensor_tensor(m3[:], x0[:, sl], sin_sb[:, sl], ALU.mult)
                        nc.vector.tensor_tensor(m4[:], x1[:, sl], cos_sb[:, sl], ALU.mult)
                        nc.vector.tensor_tensor(x0[:, sl], m1[:], m2[:], ALU.subtract)
                        nc.vector.tensor_tensor(x1[:, sl], m3[:], m4[:], ALU.add)

                    return f

                def merge_piece(t0, t1, dst):
                    # head h of pair-buffer w holds rows 64h..64h+64 =
                    # [x0_h | x1_h] via SBUF->SBUF DMA partition remap
                    def f():
                        sl = slice(ch * 512, (ch + 1) * 512)
                        for w in range(2):
                            for hh in range(2):
                                h = 2 * w + hh
                                nc.sync.dma_start(
                                    dst[w][64 * hh : 64 * hh + 32, sl],
                                    t0[32 * h : 32 * h + 32, sl],
                                )
                                nc.sync.dma_start(
                                    dst[w][64 * hh + 32 : 64 * hh + 64, sl],
                                    t1[32 * h : 32 * h + 32, sl],
                                )

                    return f

                def v_piece(st):
                    def f():
                        ps = pspool.tile([128, 256], F32, tag=ptag(), name=f"pv_{st}")
                        for dt_ in range(8):
                            nc.tensor.matmul(
                                ps[:],
                                xt_sb[:, dt_, st * 128 : (st + 1) * 128],
                                wv_sb[:, dt_, :],
                                start=(dt_ == 0),
                                stop=False,
                            )
                        nc.tensor.matmul(
                            ps[:], ones_row[0:1, :], bvr_sb[0:1, :],
                            start=False, stop=True,
                        )
                        nc.vector.tensor_copy(
                            v_sb[:, st, :].rearrange("p (h c) -> p h c", c=65)[
                                :, :, 0:64
                            ],
                            ps[:].rearrange("p (h c) -> p h c", c=64),
                        )

                    return f

                for sti in range(2):
                    pieces.append(qk_piece(q0_sb, wq0_sb, bq0_sb, sti, "q0"))
                    pieces.append(qk_piece(q1_sb, wq1_sb, bq1_sb, sti, "q1"))
                pieces.append(rope_piece(q0_sb, q1_sb))
                pieces.append(merge_piece(q0_sb, q1_sb, qm))
                for sti in range(2):
                    pieces.append(qk_piece(k0_sb, wk0_sb, bk0_sb, sti, "k0"))
                    pieces.append(qk_piece(k1_sb, wk1_sb, bk1_sb, sti, "k1"))
                pieces.append(rope_piece(k0_sb, k1_sb))
                pieces.append(merge_piece(k0_sb, k1_sb, km))
                for st in range(4 * ch, 4 * ch + 4):
                    pieces.append(v_piece(st))
                return pieces

            # ---- lead-in: chunk 0 ----
            dma_xt(0)
            for p in make_pieces(0):
                p()
            dma_xt(1)
            # Preload the ACT exp table early so the first real exp doesn't
            # pay the ~2.7us table-load stall mid-pipeline.
            warm_act = cpool.tile([128, 1], F32, tag="warm_act")
            nc.scalar.activation(warm_act[:], bq0_sb[:], AF.Exp)

            # ---- fused main loop over q-blocks ----
            for ch in range(NQB):
                pieces = make_pieces(ch + 1) if ch + 1 < NQB else []
                if ch + 2 < NQB:
                    dma_xt(ch + 2)
                nkt = 4 * ch + 4
                # pieces of chunk ch+1 interleaved into this qb's kt loop
                per_kt = -(-len(pieces) // nkt) if pieces else 0

                pvs = pspool.tile([65, HPC, QB], F32, tag="acc", name=f"pvs_{ch}")
                for kt in range(nkt):
                    j = kt - 4 * ch  # >= 0 on diagonal tiles
                    qlo = max(0, j * 128)
                    g0 = ch * 512 + qlo
                    g1 = (ch + 1) * 512
                    # two head-waves so exp(wave) overlaps PE work
                    for w in range(2):
                        psc = pspool.tile(
                            [128, 2, 512], F32, tag=f"psc{w}",
                            name=f"psc{w}_{ch}_{kt}",
                        )
                        for hh in range(2):
                            nc.tensor.matmul(
                                psc[:, hh, qlo:512],
                                km[w][
                                    64 * hh : 64 * hh + 64,
                                    kt * 128 : (kt + 1) * 128,
                                ],
                                qm[w][64 * hh : 64 * hh + 64, g0:g1],
                                start=True,
                                stop=True,
                                tile_position=(64 * hh, 0),
                            )
                        pt = ptpool.tile(
                            [128, 2, 512], BF16, tag="pt", name=f"pt{w}_{ch}_{kt}"
                        )
                        nc.scalar.activation(
                            pt[:, :, qlo:512], psc[:, :, qlo:512], AF.Exp, scale=0.125
                        )
                        if j >= 0:
                            nc.vector.tensor_tensor(
                                pt[:, :, qlo : qlo + 128],
                                pt[:, :, qlo : qlo + 128],
                                tri_b2,
                                ALU.mult,
                            )
                        for hh in range(2):
                            h = 2 * w + hh
                            nc.tensor.matmul(
                                pvs[:, h, qlo:512],
                                v_sb[:, kt, 65 * h : 65 * h + 65],
                                pt[:, hh, qlo:512],
                                start=(kt == 0),
                                stop=(kt == nkt - 1),
                            )
                    for pi in range(kt * per_kt, min((kt + 1) * per_kt, len(pieces))):
                        pieces[pi]()

                # ---- denominators -> reciprocals (per qb) ----
                nc.vector.tensor_copy(stg_sb[:], pvs[64:65, :, :])
                for h in range(HPC):
                    nc.sync.dma_start(
                        sums_sb[32 * ch + 8 * h : 32 * ch + 8 * h + 8, :],
                        stg_sb[0:1, h, :],
                    )
                nc.vector.reciprocal(
                    recip_sb[32 * ch : 32 * ch + 32, :],
                    sums_sb[32 * ch : 32 * ch + 32, :],
                )
                nc.vector.tensor_copy(
                    rb_sb[32 * ch : 32 * ch + 32, :],
                    recip_sb[32 * ch : 32 * ch + 32, :],
                )
                nc.sync.dma_start(
                    recip_d[4 * ch : 4 * ch + 4, :],
                    rb_sb[32 * ch : 32 * ch + 32, :],
                )
                # normalization fused into the PSUM drain: attended(bf16) =
                # pvs(PSUM fp32) * broadcast(1/denom); even heads land in
                # attp directly, odd heads stage then partition-remap via DMA
                for h in range(HPC):
                    bc = bcpool.tile([64, 512], BF16, tag="bc", name=f"bc_{ch}_{h}")
                    nc.sync.dma_start(
                        bc[:],
                        recip_d[4 * ch + h : 4 * ch + h + 1, :].to_broadcast(
                            (64, 512)
                        ),
                    )
                    if h % 2 == 0:
                        nc.vector.tensor_tensor(
                            attp_sb[0:64, h // 2, ch * 512 : (ch + 1) * 512],
                            pvs[0:64, h, :],
                            bc[:],
                            ALU.mult,
                        )
                    else:
                        nc.vector.tensor_tensor(
                            atth_sb[:, h // 2, :], pvs[0:64, h, :], bc[:], ALU.mult
                        )
                        nc.sync.dma_start(
                            attp_sb[64:128, h // 2, ch * 512 : (ch + 1) * 512],
                            atth_sb[:, h // 2, :],
                        )

                # ---- out projection (reuses the pv PSUM banks) ----
                for half in range(2):
                    po = pspool.tile(
                        [128, 4, 512], F32, tag="acc", name=f"po_{ch}_{half}"
                    )
                    for si in range(2):
                        sti = 2 * half + si
                        s0 = ch * 512 + sti * 128
                        for dc in range(2):
                            for t in range(2):
                                nc.tensor.matmul(
                                    po[:, 2 * si + dc, :],
                                    attp_sb[:, t, s0 : s0 + 128],
                                    wo_sb[:, t, dc * 512 : (dc + 1) * 512],
                                    start=(t == 0),
                                    stop=(t == 1),
                                )
                    for si in range(2):
                        sti = 2 * half + si
                        s0 = ch * 512 + sti * 128
                        osb_t = opool.tile(
                            [128, 2, 512], BF16, tag="osb", name=f"osb_{ch}_{sti}"
                        )
                        nc.any.tensor_copy(osb_t[:], po[:, 2 * si : 2 * si + 2, :])
                        nc.sync.dma_start(
                            partial_d[s0 : s0 + 128, :],
                            osb_t[:].rearrange("p a b -> p (a b)"),
                        )

                # ---- reduce-scatter + output (overlaps later q-blocks) ----
                nc.gpsimd.collective_compute(
                    "ReduceScatter",
                    ALU.add,
                    replica_groups=REPLICA_GROUPS,
                    ins=[partial_d[ch * 512 : (ch + 1) * 512, :]],
                    outs=[rs_d[ch][:]],
                )
                nc.gpsimd.dma_start(out_d[ch * 128 : (ch + 1) * 128, :], rs_d[ch][:])

    nc.compile()
    return nc


def make_in_maps(x, Wq, bq, Wk, bk, Wv, bv, Wo):
    inv = 1.0 / (ROPE_BASE ** (2.0 * np.arange(32, dtype=np.float64) / L))
    ang = np.arange(S, dtype=np.float64)[:, None] * inv[None, :]  # [S, 32]
    cos4 = np.tile(np.cos(ang).T, (HPC, 1)).astype(NPBF16)  # [128, S]
    sin4 = np.tile(np.sin(ang).T, (HPC, 1)).astype(NPBF16)
    tri = (np.arange(128)[None, :] >= np.arange(128)[:, None]).astype(NPBF16)

    in_maps = []
    for c in range(N_CORES):
        b, g = divmod(c, HPC)
        even = np.concatenate([64 * h + 2 * np.arange(32) for h in range(4 * g, 4 * g + 4)])
        odd = even + 1
        vcols = np.arange(256 * g, 256 * (g + 1))
        in_maps.append(
            {
                "xt": np.ascontiguousarray(x[b].T).astype(NPBF16),
                "wq0": np.ascontiguousarray(Wq[:, even]).astype(NPBF16),
                "wq1": np.ascontiguousarray(Wq[:, odd]).astype(NPBF16),
                "wk0": np.ascontiguousarray(Wk[:, even]).astype(NPBF16),
                "wk1": np.ascontiguousarray(Wk[:, odd]).astype(NPBF16),
                "wv": np.ascontiguousarray(Wv[:, vcols]).astype(NPBF16),
                "wo": np.ascontiguousarray(Wo[vcols, :]).astype(NPBF16),
                "bq0": bq[even].reshape(128, 1).astype(np.float32),
                "bq1": bq[odd].reshape(128, 1).astype(np.float32),
                "bk0": bk[even].reshape(128, 1).astype(np.float32),
                "bk1": bk[odd].reshape(128, 1).astype(np.float32),
                "bvr": bv[vcols].reshape(1, 256).astype(NPBF16),
                "cos4": cos4,
                "sin4": sin4,
                "tri": tri,
            }
        )
    return in_maps


def assemble_output(results, bo):
    out = np.empty((B, S, D), np.float32)
    for c in range(N_CORES):
        b, g = divmod(c, HPC)
        sh = np.asarray(results[c]["out"]).astype(np.float32).reshape(NQB, 128, D)
        for qb in range(NQB):
            r0 = qb * 512 + g * 128
            out[b, r0 : r0 + 128, :] = sh[qb]
    # bo is added once, after the reduction (matches `attended @ Wo + bo`).
    out += bo[None, None, :].astype(np.float32)
    return out


_CACHE = {}


def kernel(x, Wq, bq, Wk, bk, Wv, bv, Wo, bo, **run_kwargs):
    if "nc" not in _CACHE:
        _CACHE["nc"] = build_program()
    nc = _CACHE["nc"]
    in_maps = make_in_maps(
        np.asarray(x), np.asarray(Wq), np.asarray(bq), np.asarray(Wk),
        np.asarray(bk), np.asarray(Wv), np.asarray(bv), np.asarray(Wo),
    )
    res = bass_utils.run_bass_kernel_spmd(
        nc, in_maps, core_ids=list(range(N_CORES)), **run_kwargs
    )
    out = assemble_output(res.results, np.asarray(bo))
    kernel.last_results = res
    return out


# revision 7
# speedup vs baseline: 1.1144x; 1.1144x over previous
"""Trainium2 Bass kernel for causal multi-head attention with RoPE.

Problem: B=2, S=2048, D=1024, H=16 heads, L=64 head dim, causal, interleaved
RoPE, fp32 reference.

Sharding (8 cores): data-parallel over batch (2 groups of 4 cores) x tensor
parallel over heads (4 heads per core).

Fully fused pipeline per core.  Tile produces STATIC per-engine instruction
streams, so all overlap must be realized by emission interleaving:
  - QKV projection pieces for chunk ch+1 and out-projection pieces for
    q-block ch-1 are woven into attention(qb=ch)'s kt loop (proj early,
    outproj late so it never head-of-line blocks the PE stream on the
    normalization chain).
  - PV matmuls are emitted DD=4 kt's behind their scores/exp so the PE
    stream doesn't stall on the pvs PSUM bank handoff (gated by the
    previous q-block's normalization chain); pt pool is 9-deep to match.
  - ReduceScatter per q-block is issued as soon as its partials are out,
    overlapping later q-blocks on the collective engine.

PSUM (8 banks): psc0/psc1 scores double-buffer (2+2) shared with projection
and out-projection accumulators (sub-1.2us tenures so exp never starves),
plus a 4-bank pvs accumulator.

Matmuls run in bf16 (fp32 PSUM accumulation).  The RoPE pair dimension is
host-permuted into separated halves (x0 cols then x1 cols) so on-chip RoPE is
6 dense tensor ops per tensor; the permutation is applied identically to Q and
K so dot products (scores) are unchanged.
"""

import sys

import numpy as np

for _p in ("/opt/trn_rl_repo",):
    if _p not in sys.path:
        sys.path.insert(0, _p)

import ml_dtypes

import concourse.bass as bass  # noqa: F401  (registers types)
import concourse.mybir as mybir
import concourse.tile as tile
from concourse import bacc
from concourse import bass_utils

BF16 = mybir.dt.bfloat16
F32 = mybir.dt.float32
NPBF16 = ml_dtypes.bfloat16
AF = mybir.ActivationFunctionType
ALU = mybir.AluOpType

B, S, D = 2, 2048, 1024
H, L = 16, 64
HPC = 4  # heads per core
N_CORES = 8
QB = 512  # query block (columns of transposed scores)
NQB = S // QB  # 4
NKT = S // 128  # 16 key tiles
DD = 4  # PV emission deferral depth (in kt's)
ROPE_BASE = 10000.0
REPLICA_GROUPS = [[0, 1, 2, 3], [4, 5, 6, 7]]


def build_program():
    nc = bacc.Bacc(
        "TRN2", target_bir_lowering=False, debug=False, num_devices=N_CORES
    )

    # ---- I/O ----
    xt_d = nc.dram_tensor("xt", [D, S], BF16, kind="ExternalInput")
    wq0_d = nc.dram_tensor("wq0", [D, 128], BF16, kind="ExternalInput")
    wq1_d = nc.dram_tensor("wq1", [D, 128], BF16, kind="ExternalInput")
    wk0_d = nc.dram_tensor("wk0", [D, 128], BF16, kind="ExternalInput")
    wk1_d = nc.dram_tensor("wk1", [D, 128], BF16, kind="ExternalInput")
    wv_d = nc.dram_tensor("wv", [D, 256], BF16, kind="ExternalInput")
    wo_d = nc.dram_tensor("wo", [256, D], BF16, kind="ExternalInput")
    bq0_d = nc.dram_tensor("bq0", [128, 1], F32, kind="ExternalInput")
    bq1_d = nc.dram_tensor("bq1", [128, 1], F32, kind="ExternalInput")
    bk0_d = nc.dram_tensor("bk0", [128, 1], F32, kind="ExternalInput")
    bk1_d = nc.dram_tensor("bk1", [128, 1], F32, kind="ExternalInput")
    bvr_d = nc.dram_tensor("bvr", [1, 256], BF16, kind="ExternalInput")
    cos_d = nc.dram_tensor("cos4", [128, S], BF16, kind="ExternalInput")
    sin_d = nc.dram_tensor("sin4", [128, S], BF16, kind="ExternalInput")
    tri_d = nc.dram_tensor("tri", [128, 128], BF16, kind="ExternalInput")
    out_d = nc.dram_tensor("out", [4 * 128, D], BF16, kind="ExternalOutput")

    partial_d = nc.dram_tensor("partial", [S, D], BF16, kind="Internal")
    recip_d = nc.dram_tensor("recipd", [4 * HPC, 512], BF16, kind="Internal")
    rs_d = [
        nc.dram_tensor(f"rs{qb}", [128, D], BF16, kind="Internal")
        for qb in range(NQB)
    ]

    with tile.TileContext(nc) as tc:
        with (
            tc.tile_pool(name="const", bufs=1) as cpool,
            tc.tile_pool(name="xp", bufs=1) as xpool,
            tc.tile_pool(name="qk", bufs=1) as qkpool,
            tc.tile_pool(name="rtmp", bufs=1) as rtmp,
            tc.tile_pool(name="ptp", bufs=9) as ptpool,
            tc.tile_pool(name="att", bufs=1) as attpool,
            tc.tile_pool(name="bc", bufs=2) as bcpool,
            tc.tile_pool(name="osb", bufs=3) as opool,
            tc.tile_pool(name="ps", bufs=1, space="PSUM") as pspool,
        ):
            # ---- lead-in DMAs, ordered for earliest PE start ----
            # bq0 first (feeds warm_act), then xt chunk 0 + wq on the two
            # HWDGE queues in parallel; remaining consts follow.
            bq0_sb = cpool.tile([128, 1], F32, tag="bq0")
            nc.scalar.dma_start(bq0_sb[:], bq0_d.ap())
            # Preload the ACT exp table so the first real exp doesn't pay
            # the ~2.7us table-load stall mid-pipeline.
            warm_act = cpool.tile([128, 1], F32, tag="warm_act")
            nc.scalar.activation(warm_act[:], bq0_sb[:], AF.Exp)

            xt_sb = xpool.tile([128, 8, S], BF16)
            xt_r = xt_d.ap().rearrange("(o p) s -> p o s", p=128)

            def dma_xt(ch):
                for dt_ in range(8):
                    nc.sync.dma_start(
                        xt_sb[:, dt_, ch * 512 : (ch + 1) * 512],
                        xt_r[:, dt_, ch * 512 : (ch + 1) * 512],
                    )

            dma_xt(0)

            def load_w(dram, cols):
                t = cpool.tile([128, 8, cols], BF16, tag=f"w_{dram.name}")
                nc.scalar.dma_start(
                    t[:], dram.ap().rearrange("(o p) m -> p o m", p=128)
                )
                return t

            def load_c(dram, shape, dt, tag, eng=None):
                t = cpool.tile(shape, dt, tag=tag)
                (eng or nc.scalar).dma_start(t[:], dram.ap())
                return t

            wq0_sb = load_w(wq0_d, 128)
            wq1_sb = load_w(wq1_d, 128)
            bq1_sb = load_c(bq1_d, [128, 1], F32, "bq1")
            wk0_sb = load_w(wk0_d, 128)
            wk1_sb = load_w(wk1_d, 128)
            bk0_sb = load_c(bk0_d, [128, 1], F32, "bk0")
            bk1_sb = load_c(bk1_d, [128, 1], F32, "bk1")
            cos_sb = load_c(cos_d, [128, S], BF16, "cos4", nc.sync)
            sin_sb = load_c(sin_d, [128, S], BF16, "sin4", nc.sync)
            wv_sb = load_w(wv_d, 256)
            bvr_sb = load_c(bvr_d, [1, 256], BF16, "bvr")
            tri_sb = load_c(tri_d, [128, 128], BF16, "tri", nc.sync)
            wo_sb = cpool.tile([128, 2, D], BF16)
            nc.scalar.dma_start(
                wo_sb[:], wo_d.ap().rearrange("(o p) m -> p o m", p=128)
            )

            ones_row = cpool.tile([1, 128], BF16, tag="ones_row")
            nc.vector.memset(ones_row[:], 1.0)

            # ---- persistent SBUF state ----
            q0_sb = qkpool.tile([128, S], BF16, tag="q0")
            q1_sb = qkpool.tile([128, S], BF16, tag="q1")
            k0_sb = qkpool.tile([128, S], BF16, tag="k0")
            k1_sb = qkpool.tile([128, S], BF16, tag="k1")
            qm = [
                qkpool.tile([128, S], BF16, tag=f"qm{w}", name=f"qm{w}")
                for w in range(2)
            ]
            km = [
                qkpool.tile([128, S], BF16, tag=f"km{w}", name=f"km{w}")
                for w in range(2)
            ]
            # V with a ones column appended per head (65 cols/head): the PV
            # matmul then emits the softmax denominator in output row 64.
            v_sb = qkpool.tile([128, NKT, HPC * 65], BF16, tag="v")
            nc.vector.memset(
                v_sb[:].rearrange("p t (h c) -> p t h c", c=65)[:, :, :, 64:65], 1.0
            )

            atth_sb = attpool.tile([64, 2, QB], BF16, tag="atth")  # odd-head stage
            attp_sb = attpool.tile([128, 2, S], BF16, tag="attp")
            sums_sb = attpool.tile([128, 64], F32, tag="sums")
            recip_sb = attpool.tile([128, 64], F32, tag="recip")
            stg_sb = attpool.tile([65, HPC, QB], F32, tag="stg")
            rb_sb = attpool.tile([128, 64], BF16, tag="rb")

            tri_b2 = tri_sb[:, None, :].to_broadcast((128, 2, 128))

            # psc-tag alternation shared by scores, projection pieces and
            # outproj pieces: every tenure is short enough that the exp
            # stream (alternating psc0/psc1) never starves.
            _ptag = [0]

            def ptag():
                _ptag[0] ^= 1
                return f"psc{_ptag[0]}"

            # ---- projection pieces for one 512-seq chunk ----
            def make_pieces(ch):
                pieces = []

                def qk_piece(dst, w_sb, b_sb, sti, nm):
                    def f():
                        c0 = ch * 512 + sti * 256
                        ps = pspool.tile(
                            [128, 256], F32, tag=ptag(), name=f"pp_{nm}_{ch}_{sti}"
                        )
                        for dt_ in range(8):
                            nc.tensor.matmul(
                                ps[:],
                                w_sb[:, dt_, :],
                                xt_sb[:, dt_, c0 : c0 + 256],
                                start=(dt_ == 0),
                                stop=(dt_ == 7),
                            )
                        nc.vector.tensor_scalar(
                            dst[:, c0 : c0 + 256], ps[:], b_sb[:, 0:1], None, ALU.add
                        )

                    return f

                def rope_piece(x0, x1, eng):
                    def f():
                        sl = slice(ch * 512, (ch + 1) * 512)
                        m1 = rtmp.tile([128, 512], BF16, tag="m1")
                        m2 = rtmp.tile([128, 512], BF16, tag="m2")
                        m3 = rtmp.tile([128, 512], BF16, tag="m3")
                        m4 = rtmp.tile([128, 512], BF16, tag="m4")
                        eng.tensor_tensor(m1[:], x0[:, sl], cos_sb[:, sl], ALU.mult)
                        eng.tensor_tensor(m2[:], x1[:, sl], sin_sb[:, sl], ALU.mult)
                        eng.tensor_tensor(m3[:], x0[:, sl], sin_sb[:, sl], ALU.mult)
                        eng.tensor_tensor(m4[:], x1[:, sl], cos_sb[:, sl], ALU.mult)
                        eng.tensor_tensor(x0[:, sl], m1[:], m2[:], ALU.subtract)
                        eng.tensor_tensor(x1[:, sl], m3[:], m4[:], ALU.add)

                    return f

                def merge_piece(t0, t1, dst):
                    # head h of pair-buffer w holds rows 64h..64h+64 =
                    # [x0_h | x1_h] via SBUF->SBUF DMA partition remap
                    def f():
                        sl = slice(ch * 512, (ch + 1) * 512)
                        for w in range(2):
                            for hh in range(2):
                                h = 2 * w + hh
                                nc.sync.dma_start(
                                    dst[w][64 * hh : 64 * hh + 32, sl],
                                    t0[32 * h : 32 * h + 32, sl],
                                )
                                nc.sync.dma_start(
                                    dst[w][64 * hh + 32 : 64 * hh + 64, sl],
                                    t1[32 * h : 32 * h + 32, sl],
                                )

                    return f

                def v_piece(st):
                    def f():
                        ps = pspool.tile([128, 256], F32, tag=ptag(), name=f"pv_{st}")
                        for dt_ in range(8):
                            nc.tensor.matmul(
                                ps[:],
                                xt_sb[:, dt_, st * 128 : (st + 1) * 128],
                                wv_sb[:, dt_, :],
                                start=(dt_ == 0),
                                stop=False,
                            )
                        nc.tensor.matmul(
                            ps[:], ones_row[0:1, :], bvr_sb[0:1, :],
                            start=False, stop=True,
                        )
                        nc.vector.tensor_copy(
                            v_sb[:, st, :].rearrange("p (h c) -> p h c", c=65)[
                                :, :, 0:64
                            ],
                            ps[:].rearrange("p (h c) -> p h c", c=64),
                        )

                    return f

                for sti in range(2):
                    pieces.append(qk_piece(q0_sb, wq0_sb, bq0_sb, sti, "q0"))
                    pieces.append(qk_piece(q1_sb, wq1_sb, bq1_sb, sti, "q1"))
                pieces.append(rope_piece(q0_sb, q1_sb, nc.vector))
                pieces.append(merge_piece(q0_sb, q1_sb, qm))
                for sti in range(2):
                    pieces.append(qk_piece(k0_sb, wk0_sb, bk0_sb, sti, "k0"))
                    pieces.append(qk_piece(k1_sb, wk1_sb, bk1_sb, sti, "k1"))
                pieces.append(rope_piece(k0_sb, k1_sb, nc.gpsimd))
                pieces.append(merge_piece(k0_sb, k1_sb, km))
                for st in range(4 * ch, 4 * ch + 4):
                    pieces.append(v_piece(st))
                return pieces

            # ---- out-projection pieces for one finished q-block ----
            def make_outproj_pieces(ch, drain_eng):
                pieces = []
                osb_box = [None]

                def po_piece(sti, dc):
                    def f():
                        s0 = ch * 512 + sti * 128
                        if dc == 0:
                            osb_box[0] = opool.tile(
                                [128, 2, 512], BF16, tag="osb",
                                name=f"osb_{ch}_{sti}",
                            )
                        po = pspool.tile(
                            [128, 512], F32, tag=ptag(), name=f"po_{ch}_{sti}_{dc}"
                        )
                        for t in range(2):
                            nc.tensor.matmul(
                                po[:],
                                attp_sb[:, t, s0 : s0 + 128],
                                wo_sb[:, t, dc * 512 : (dc + 1) * 512],
                                start=(t == 0),
                                stop=(t == 1),
                            )
                        if drain_eng is nc.scalar:
                            nc.scalar.activation(
                                osb_box[0][:, dc, :], po[:], AF.Copy
                            )
                        else:
                            nc.vector.tensor_copy(osb_box[0][:, dc, :], po[:])
                        if dc == 1:
                            nc.sync.dma_start(
                                partial_d[s0 : s0 + 128, :],
                                osb_box[0][:].rearrange("p a b -> p (a b)"),
                            )

                    return f

                for sti in range(4):
                    for dc in range(2):
                        pieces.append(po_piece(sti, dc))

                def rs_piece():
                    nc.gpsimd.collective_compute(
                        "ReduceScatter",
                        ALU.add,
                        replica_groups=REPLICA_GROUPS,
                        ins=[partial_d[ch * 512 : (ch + 1) * 512, :]],
                        outs=[rs_d[ch][:]],
                    )
                    nc.gpsimd.dma_start(
                        out_d[ch * 128 : (ch + 1) * 128, :], rs_d[ch][:]
                    )

                pieces.append(rs_piece)
                return pieces

            # ---- per-q-block normalization chain ----
            def norm_block(ch, pvs):
                # denominators (pvs row 64) -> packed reciprocals -> DRAM ->
                # partition-broadcast -> fused normalize+drain of attended
                for h in range(HPC):
                    nc.vector.tensor_copy(stg_sb[64:65, h, :], pvs[64:65, h, :])
                    nc.sync.dma_start(
                        sums_sb[32 * ch + 8 * h : 32 * ch + 8 * h + 8, :],
                        stg_sb[64:65, h, :],
                    )
                nc.vector.reciprocal(
                    recip_sb[32 * ch : 32 * ch + 32, :],
                    sums_sb[32 * ch : 32 * ch + 32, :],
                )
                nc.vector.tensor_copy(
                    rb_sb[32 * ch : 32 * ch + 32, :],
                    recip_sb[32 * ch : 32 * ch + 32, :],
                )
                nc.sync.dma_start(
                    recip_d[4 * ch : 4 * ch + 4, :],
                    rb_sb[32 * ch : 32 * ch + 32, :],
                )
                for h in range(HPC):
                    bc = bcpool.tile([64, 512], BF16, tag="bc", name=f"bc_{ch}_{h}")
                    nc.sync.dma_start(
                        bc[:],
                        recip_d[4 * ch + h : 4 * ch + h + 1, :].to_broadcast(
                            (64, 512)
                        ),
                    )
                    if h % 2 == 0:
                        nc.vector.tensor_tensor(
                            attp_sb[0:64, h // 2, ch * 512 : (ch + 1) * 512],
                            pvs[0:64, h, :],
                            bc[:],
                            ALU.mult,
                        )
                    else:
                        nc.vector.tensor_tensor(
                            atth_sb[:, h // 2, :], pvs[0:64, h, :], bc[:], ALU.mult
                        )
                        nc.sync.dma_start(
                            attp_sb[64:128, h // 2, ch * 512 : (ch + 1) * 512],
                            atth_sb[:, h // 2, :],
                        )

            # ---- emit: lead-in pieces, then the fused q-block loop ----
            for p in make_pieces(0):
                p()
            dma_xt(1)

            for ch in range(NQB):
                nkt = 4 * ch + 4
                if ch + 2 < NQB:
                    dma_xt(ch + 2)
                # work hosted inside this qb's kt loop
                early = make_pieces(ch + 1) if ch + 1 < NQB else []
                late = make_outproj_pieces(ch - 1, nc.vector) if ch >= 1 else []
                work = {}
                if early:
                    k1 = max(1, nkt - 3)
                    for i, it in enumerate(early):
                        work.setdefault(i * k1 // len(early), []).append(it)
                if late:
                    lo = min(5, nkt - 3)
                    hi = min(nkt, lo + 4)
                    for i, it in enumerate(late):
                        work.setdefault(
                            lo + i * (hi - lo) // len(late), []
                        ).append(it)

                pvs = pspool.tile([65, HPC, QB], F32, tag="acc", name=f"pvs_{ch}")
                pv_stash = []
                for kt in range(nkt):
                    j = kt - 4 * ch  # >= 0 on diagonal tiles
                    qlo = max(0, j * 128)
                    g0 = ch * 512 + qlo
                    g1 = (ch + 1) * 512
                    for w in range(2):
                        psc = pspool.tile(
                            [128, 2, 512], F32, tag=f"psc{w}",
                            name=f"psc{w}_{ch}_{kt}",
                        )
                        for hh in range(2):
                            nc.tensor.matmul(
                                psc[:, hh, qlo:512],
                                km[w][
                                    64 * hh : 64 * hh + 64,
                                    kt * 128 : (kt + 1) * 128,
                                ],
                                qm[w][64 * hh : 64 * hh + 64, g0:g1],
                                start=True,
                                stop=True,
                                tile_position=(64 * hh, 0),
                            )
                        pt = ptpool.tile(
                            [128, 2, 512], BF16, tag="pt", name=f"pt{w}_{ch}_{kt}"
                        )
                        nc.scalar.activation(
                            pt[:, :, qlo:512], psc[:, :, qlo:512], AF.Exp, scale=0.125
                        )
                        if j >= 0:
                            nc.vector.tensor_tensor(
                                pt[:, :, qlo : qlo + 128],
                                pt[:, :, qlo : qlo + 128],
                                tri_b2,
                                ALU.mult,
                            )

                        def pv_emit(kt=kt, w=w, pt=pt, qlo=qlo):
                            for hh in range(2):
                                h = 2 * w + hh
                                nc.tensor.matmul(
                                    pvs[:, h, qlo:512],
                                    v_sb[:, kt, 65 * h : 65 * h + 65],
                                    pt[:, hh, qlo:512],
                                    start=(kt == 0),
                                    stop=(kt == nkt - 1),
                                )

                        pv_stash.append(pv_emit)
                    # deferred PV emission (DD kt's behind scores/exp)
                    while len(pv_stash) > 2 * DD:
                        pv_stash.pop(0)()
                    for it in work.get(kt, []):
                        it()
                for f in pv_stash:
                    f()
                pv_stash.clear()

                norm_block(ch, pvs)

            # last q-block's out-projection + RS (ACT is idle by now)
            for p in make_outproj_pieces(NQB - 1, nc.scalar):
                p()

    nc.compile()
    return nc


def make_in_maps(x, Wq, bq, Wk, bk, Wv, bv, Wo):
    inv = 1.0 / (ROPE_BASE ** (2.0 * np.arange(32, dtype=np.float64) / L))
    ang = np.arange(S, dtype=np.float64)[:, None] * inv[None, :]  # [S, 32]
    cos4 = np.tile(np.cos(ang).T, (HPC, 1)).astype(NPBF16)  # [128, S]
    sin4 = np.tile(np.sin(ang).T, (HPC, 1)).astype(NPBF16)
    tri = (np.arange(128)[None, :] >= np.arange(128)[:, None]).astype(NPBF16)

    in_maps = []
    for c in range(N_CORES):
        b, g = divmod(c, HPC)
        even = np.concatenate([64 * h + 2 * np.arange(32) for h in range(4 * g, 4 * g + 4)])
        odd = even + 1
        vcols = np.arange(256 * g, 256 * (g + 1))
        in_maps.append(
            {
                "xt": np.ascontiguousarray(x[b].T).astype(NPBF16),
                "wq0": np.ascontiguousarray(Wq[:, even]).astype(NPBF16),
                "wq1": np.ascontiguousarray(Wq[:, odd]).astype(NPBF16),
                "wk0": np.ascontiguousarray(Wk[:, even]).astype(NPBF16),
                "wk1": np.ascontiguousarray(Wk[:, odd]).astype(NPBF16),
                "wv": np.ascontiguousarray(Wv[:, vcols]).astype(NPBF16),
                "wo": np.ascontiguousarray(Wo[vcols, :]).astype(NPBF16),
                "bq0": bq[even].reshape(128, 1).astype(np.float32),
                "bq1": bq[odd].reshape(128, 1).astype(np.float32),
                "bk0": bk[even].reshape(128, 1).astype(np.float32),
                "bk1": bk[odd].reshape(128, 1).astype(np.float32),
                "bvr": bv[vcols].reshape(1, 256).astype(NPBF16),
                "cos4": cos4,
                "sin4": sin4,
                "tri": tri,
            }
        )
    return in_maps


def assemble_output(results, bo):
    out = np.empty((B, S, D), np.float32)
    for c in range(N_CORES):
        b, g = divmod(c, HPC)
        sh = np.asarray(results[c]["out"]).astype(np.float32).reshape(NQB, 128, D)
        for qb in range(NQB):
            r0 = qb * 512 + g * 128
            out[b, r0 : r0 + 128, :] = sh[qb]
    # bo is added once, after the reduction (matches `attended @ Wo + bo`).
    out += bo[None, None, :].astype(np.float32)
    return out


_CACHE = {}


def kernel(x, Wq, bq, Wk, bk, Wv, bv, Wo, bo, **run_kwargs):
    if "nc" not in _CACHE:
        _CACHE["nc"] = build_program()
    nc = _CACHE["nc"]
    in_maps = make_in_maps(
        np.asarray(x), np.asarray(Wq), np.asarray(bq), np.asarray(Wk),
        np.asarray(bk), np.asarray(Wv), np.asarray(bv), np.asarray(Wo),
    )
    res = bass_utils.run_bass_kernel_spmd(
        nc, in_maps, core_ids=list(range(N_CORES)), **run_kwargs
    )
    out = assemble_output(res.results, np.asarray(bo))
    kernel.last_results = res
    return out


# revision 8
# speedup vs baseline: 1.1545x; 1.0360x over previous
"""Trainium2 Bass kernel for causal multi-head attention with RoPE.

Problem: B=2, S=2048, D=1024, H=16 heads, L=64 head dim, causal, interleaved
RoPE, fp32 reference.

Sharding (8 cores): data-parallel over batch (2 groups of 4 cores) x tensor
parallel over heads (4 heads per core).

Fully fused pipeline per core.  Tile produces STATIC per-engine instruction
streams, so all overlap must be realized by emission interleaving:
  - QKV projection pieces for chunk ch+1 and out-projection pieces for
    q-block ch-1 are woven into attention(qb=ch)'s kt loop (proj early,
    outproj late so it never head-of-line blocks the PE stream on the
    normalization chain).
  - PV matmuls are emitted DD=4 kt's behind their scores/exp so the PE
    stream doesn't stall on the pvs PSUM bank handoff (gated by the
    previous q-block's normalization chain); pt pool is 9-deep to match.
  - ReduceScatter per q-block is issued as soon as its partials are out,
    overlapping later q-blocks on the collective engine.

PSUM (8 banks): psc0/psc1 scores double-buffer (2+2) shared with projection
and out-projection accumulators (sub-1.2us tenures so exp never starves),
plus a 4-bank pvs accumulator.

Matmuls run in bf16 (fp32 PSUM accumulation).  The RoPE pair dimension is
host-permuted into separated halves (x0 cols then x1 cols) so on-chip RoPE is
6 dense tensor ops per tensor; the permutation is applied identically to Q and
K so dot products (scores) are unchanged.
"""

import sys

import numpy as np

for _p in ("/opt/trn_rl_repo",):
    if _p not in sys.path:
        sys.path.insert(0, _p)

import ml_dtypes

import concourse.bass as bass  # noqa: F401  (registers types)
import concourse.mybir as mybir
import concourse.tile as tile
from concourse import bacc
from concourse import bass_utils

BF16 = mybir.dt.bfloat16
F32 = mybir.dt.float32
NPBF16 = ml_dtypes.bfloat16
AF = mybir.ActivationFunctionType
ALU = mybir.AluOpType

B, S, D = 2, 2048, 1024
H, L = 16, 64
HPC = 4  # heads per core
N_CORES = 8
QB = 512  # query block (columns of transposed scores)
NQB = S // QB  # 4
NKT = S // 128  # 16 key tiles
DD = 4  # PV emission deferral depth (in kt's)
ROPE_BASE = 10000.0
REPLICA_GROUPS = [[0, 1, 2, 3], [4, 5, 6, 7]]


def build_program():
    nc = bacc.Bacc(
        "TRN2", target_bir_lowering=False, debug=False, num_devices=N_CORES
    )

    # ---- I/O ----
    xt_d = nc.dram_tensor("xt", [D, S], BF16, kind="ExternalInput")
    wq0_d = nc.dram_tensor("wq0", [D, 128], BF16, kind="ExternalInput")
    wq1_d = nc.dram_tensor("wq1", [D, 128], BF16, kind="ExternalInput")
    wk0_d = nc.dram_tensor("wk0", [D, 128], BF16, kind="ExternalInput")
    wk1_d = nc.dram_tensor("wk1", [D, 128], BF16, kind="ExternalInput")
    wv_d = nc.dram_tensor("wv", [D, 256], BF16, kind="ExternalInput")
    wo_d = nc.dram_tensor("wo", [256, D], BF16, kind="ExternalInput")
    bq0_d = nc.dram_tensor("bq0", [128, 1], F32, kind="ExternalInput")
    bq1_d = nc.dram_tensor("bq1", [128, 1], F32, kind="ExternalInput")
    bk0_d = nc.dram_tensor("bk0", [128, 1], F32, kind="ExternalInput")
    bk1_d = nc.dram_tensor("bk1", [128, 1], F32, kind="ExternalInput")
    bvr_d = nc.dram_tensor("bvr", [1, 256], BF16, kind="ExternalInput")
    cos_d = nc.dram_tensor("cos4", [128, S], BF16, kind="ExternalInput")
    sin_d = nc.dram_tensor("sin4", [128, S], BF16, kind="ExternalInput")
    tri_d = nc.dram_tensor("tri", [128, 128], BF16, kind="ExternalInput")
    out_d = nc.dram_tensor("out", [4 * 128, D], BF16, kind="ExternalOutput")

    partial_d = nc.dram_tensor("partial", [S, D], BF16, kind="Internal")
    recip_d = nc.dram_tensor("recipd", [4 * HPC, 512], BF16, kind="Internal")
    rs_d = [
        nc.dram_tensor(f"rs{qb}", [128, D], BF16, kind="Internal")
        for qb in range(NQB)
    ]

    with tile.TileContext(nc) as tc:
        with (
            tc.tile_pool(name="const", bufs=1) as cpool,
            tc.tile_pool(name="xp", bufs=1) as xpool,
            tc.tile_pool(name="qk", bufs=1) as qkpool,
            tc.tile_pool(name="rtmp", bufs=1) as rtmp,
            tc.tile_pool(name="ptp", bufs=9) as ptpool,
            tc.tile_pool(name="att", bufs=1) as attpool,
            tc.tile_pool(name="bc", bufs=2) as bcpool,
            tc.tile_pool(name="osb", bufs=3) as opool,
            tc.tile_pool(name="ps", bufs=1, space="PSUM") as pspool,
        ):
            # ---- lead-in DMAs, ordered for earliest PE start ----
            # bq0 first (feeds warm_act), then xt chunk 0 + wq on the two
            # HWDGE queues in parallel; remaining consts follow.
            bq0_sb = cpool.tile([128, 1], F32, tag="bq0")
            nc.scalar.dma_start(bq0_sb[:], bq0_d.ap())
            # Preload the ACT exp table so the first real exp doesn't pay
            # the ~2.7us table-load stall mid-pipeline.
            warm_act = cpool.tile([128, 1], F32, tag="warm_act")
            nc.scalar.activation(warm_act[:], bq0_sb[:], AF.Exp)

            xt_sb = xpool.tile([128, 8, S], BF16)
            xt_r = xt_d.ap().rearrange("(o p) s -> p o s", p=128)

            def dma_xt(ch):
                for dt_ in range(8):
                    nc.sync.dma_start(
                        xt_sb[:, dt_, ch * 512 : (ch + 1) * 512],
                        xt_r[:, dt_, ch * 512 : (ch + 1) * 512],
                    )

            dma_xt(0)

            def load_w(dram, cols):
                t = cpool.tile([128, 8, cols], BF16, tag=f"w_{dram.name}")
                nc.scalar.dma_start(
                    t[:], dram.ap().rearrange("(o p) m -> p o m", p=128)
                )
                return t

            def load_c(dram, shape, dt, tag, eng=None):
                t = cpool.tile(shape, dt, tag=tag)
                (eng or nc.scalar).dma_start(t[:], dram.ap())
                return t

            wq0_sb = load_w(wq0_d, 128)
            wq1_sb = load_w(wq1_d, 128)
            bq1_sb = load_c(bq1_d, [128, 1], F32, "bq1")
            wk0_sb = load_w(wk0_d, 128)
            wk1_sb = load_w(wk1_d, 128)
            bk0_sb = load_c(bk0_d, [128, 1], F32, "bk0")
            bk1_sb = load_c(bk1_d, [128, 1], F32, "bk1")
            cos_sb = load_c(cos_d, [128, S], BF16, "cos4", nc.sync)
            sin_sb = load_c(sin_d, [128, S], BF16, "sin4", nc.sync)
            wv_sb = load_w(wv_d, 256)
            bvr_sb = load_c(bvr_d, [1, 256], BF16, "bvr")
            tri_sb = load_c(tri_d, [128, 128], BF16, "tri", nc.sync)
            wo_sb = cpool.tile([128, 2, D], BF16)
            nc.scalar.dma_start(
                wo_sb[:], wo_d.ap().rearrange("(o p) m -> p o m", p=128)
            )

            ones_row = cpool.tile([1, 128], BF16, tag="ones_row")
            nc.vector.memset(ones_row[:], 1.0)

            # ---- persistent SBUF state ----
            q0_sb = qkpool.tile([128, S], BF16, tag="q0")
            q1_sb = qkpool.tile([128, S], BF16, tag="q1")
            k0_sb = qkpool.tile([128, S], BF16, tag="k0")
            k1_sb = qkpool.tile([128, S], BF16, tag="k1")
            qm = [
                qkpool.tile([128, S], BF16, tag=f"qm{w}", name=f"qm{w}")
                for w in range(2)
            ]
            km = [
                qkpool.tile([128, S], BF16, tag=f"km{w}", name=f"km{w}")
                for w in range(2)
            ]
            # V with a ones column appended per head (65 cols/head): the PV
            # matmul then emits the softmax denominator in output row 64.
            v_sb = qkpool.tile([128, NKT, HPC * 65], BF16, tag="v")
            nc.vector.memset(
                v_sb[:].rearrange("p t (h c) -> p t h c", c=65)[:, :, :, 64:65], 1.0
            )

            atth_sb = attpool.tile([64, 2, QB], BF16, tag="atth")  # odd-head stage
            attp_sb = attpool.tile([128, 2, S], BF16, tag="attp")
            sums_sb = attpool.tile([128, 64], F32, tag="sums")
            recip_sb = attpool.tile([128, 64], F32, tag="recip")
            stg_sb = attpool.tile([65, HPC, QB], F32, tag="stg")
            rb_sb = attpool.tile([128, 64], BF16, tag="rb")

            tri_b2 = tri_sb[:, None, :].to_broadcast((128, 2, 128))

            # psc-tag alternation shared by scores, projection pieces and
            # outproj pieces: every tenure is short enough that the exp
            # stream (alternating psc0/psc1) never starves.
            _ptag = [0]

            def ptag():
                _ptag[0] ^= 1
                return f"psc{_ptag[0]}"

            # ---- projection pieces for one 512-seq chunk ----
            def make_pieces(ch):
                pieces = []

                def qk_piece(dst, w_sb, b_sb, sti, nm):
                    def f():
                        c0 = ch * 512 + sti * 256
                        ps = pspool.tile(
                            [128, 256], F32, tag=ptag(), name=f"pp_{nm}_{ch}_{sti}"
                        )
                        for dt_ in range(8):
                            nc.tensor.matmul(
                                ps[:],
                                w_sb[:, dt_, :],
                                xt_sb[:, dt_, c0 : c0 + 256],
                                start=(dt_ == 0),
                                stop=(dt_ == 7),
                            )
                        nc.vector.tensor_scalar(
                            dst[:, c0 : c0 + 256], ps[:], b_sb[:, 0:1], None, ALU.add
                        )

                    return f

                def rope_piece(x0, x1, eng):
                    def f():
                        sl = slice(ch * 512, (ch + 1) * 512)
                        m1 = rtmp.tile([128, 512], BF16, tag="m1")
                        m2 = rtmp.tile([128, 512], BF16, tag="m2")
                        m3 = rtmp.tile([128, 512], BF16, tag="m3")
                        m4 = rtmp.tile([128, 512], BF16, tag="m4")
                        eng.tensor_tensor(m1[:], x0[:, sl], cos_sb[:, sl], ALU.mult)
                        eng.tensor_tensor(m2[:], x1[:, sl], sin_sb[:, sl], ALU.mult)
                        eng.tensor_tensor(m3[:], x0[:, sl], sin_sb[:, sl], ALU.mult)
                        eng.tensor_tensor(m4[:], x1[:, sl], cos_sb[:, sl], ALU.mult)
                        eng.tensor_tensor(x0[:, sl], m1[:], m2[:], ALU.subtract)
                        eng.tensor_tensor(x1[:, sl], m3[:], m4[:], ALU.add)

                    return f

                def merge_piece(t0, t1, dst):
                    # head h of pair-buffer w holds rows 64h..64h+64 =
                    # [x0_h | x1_h] via SBUF->SBUF DMA partition remap
                    def f():
                        sl = slice(ch * 512, (ch + 1) * 512)
                        for w in range(2):
                            for hh in range(2):
                                h = 2 * w + hh
                                nc.sync.dma_start(
                                    dst[w][64 * hh : 64 * hh + 32, sl],
                                    t0[32 * h : 32 * h + 32, sl],
                                )
                                nc.sync.dma_start(
                                    dst[w][64 * hh + 32 : 64 * hh + 64, sl],
                                    t1[32 * h : 32 * h + 32, sl],
                                )

                    return f

                def v_piece(st):
                    def f():
                        ps = pspool.tile([128, 256], F32, tag=ptag(), name=f"pv_{st}")
                        for dt_ in range(8):
                            nc.tensor.matmul(
                                ps[:],
                                xt_sb[:, dt_, st * 128 : (st + 1) * 128],
                                wv_sb[:, dt_, :],
                                start=(dt_ == 0),
                                stop=False,
                            )
                        nc.tensor.matmul(
                            ps[:], ones_row[0:1, :], bvr_sb[0:1, :],
                            start=False, stop=True,
                        )
                        nc.vector.tensor_copy(
                            v_sb[:, st, :].rearrange("p (h c) -> p h c", c=65)[
                                :, :, 0:64
                            ],
                            ps[:].rearrange("p (h c) -> p h c", c=64),
                        )

                    return f

                for sti in range(2):
                    pieces.append(qk_piece(q0_sb, wq0_sb, bq0_sb, sti, "q0"))
                    pieces.append(qk_piece(q1_sb, wq1_sb, bq1_sb, sti, "q1"))
                pieces.append(rope_piece(q0_sb, q1_sb, nc.vector))
                pieces.append(merge_piece(q0_sb, q1_sb, qm))
                for sti in range(2):
                    pieces.append(qk_piece(k0_sb, wk0_sb, bk0_sb, sti, "k0"))
                    pieces.append(qk_piece(k1_sb, wk1_sb, bk1_sb, sti, "k1"))
                pieces.append(rope_piece(k0_sb, k1_sb, nc.vector))
                pieces.append(merge_piece(k0_sb, k1_sb, km))
                for st in range(4 * ch, 4 * ch + 4):
                    pieces.append(v_piece(st))
                return pieces

            # ---- out-projection pieces for one finished q-block ----
            def make_outproj_pieces(ch, drain_eng):
                pieces = []
                osb_box = [None]

                def po_piece(sti, dc):
                    def f():
                        s0 = ch * 512 + sti * 128
                        if dc == 0:
                            osb_box[0] = opool.tile(
                                [128, 2, 512], BF16, tag="osb",
                                name=f"osb_{ch}_{sti}",
                            )
                        po = pspool.tile(
                            [128, 512], F32, tag=ptag(), name=f"po_{ch}_{sti}_{dc}"
                        )
                        for t in range(2):
                            nc.tensor.matmul(
                                po[:],
                                attp_sb[:, t, s0 : s0 + 128],
                                wo_sb[:, t, dc * 512 : (dc + 1) * 512],
                                start=(t == 0),
                                stop=(t == 1),
                            )
                        if drain_eng is nc.scalar:
                            nc.scalar.activation(
                                osb_box[0][:, dc, :], po[:], AF.Copy
                            )
                        else:
                            nc.vector.tensor_copy(osb_box[0][:, dc, :], po[:])
                        if dc == 1:
                            nc.sync.dma_start(
                                partial_d[s0 : s0 + 128, :],
                                osb_box[0][:].rearrange("p a b -> p (a b)"),
                            )

                    return f

                for sti in range(4):
                    for dc in range(2):
                        pieces.append(po_piece(sti, dc))

                def rs_piece():
                    nc.gpsimd.collective_compute(
                        "ReduceScatter",
                        ALU.add,
                        replica_groups=REPLICA_GROUPS,
                        ins=[partial_d[ch * 512 : (ch + 1) * 512, :]],
                        outs=[rs_d[ch][:]],
                    )
                    nc.gpsimd.dma_start(
                        out_d[ch * 128 : (ch + 1) * 128, :], rs_d[ch][:]
                    )

                pieces.append(rs_piece)
                return pieces

            # ---- per-q-block normalization chain ----
            def norm_block(ch, pvs):
                # denominators (pvs row 64) -> packed reciprocals -> DRAM ->
                # partition-broadcast -> fused normalize+drain of attended
                for h in range(HPC):
                    nc.vector.tensor_copy(stg_sb[64:65, h, :], pvs[64:65, h, :])
                    nc.sync.dma_start(
                        sums_sb[32 * ch + 8 * h : 32 * ch + 8 * h + 8, :],
                        stg_sb[64:65, h, :],
                    )
                nc.vector.reciprocal(
                    recip_sb[32 * ch : 32 * ch + 32, :],
                    sums_sb[32 * ch : 32 * ch + 32, :],
                )
                nc.vector.tensor_copy(
                    rb_sb[32 * ch : 32 * ch + 32, :],
                    recip_sb[32 * ch : 32 * ch + 32, :],
                )
                nc.sync.dma_start(
                    recip_d[4 * ch : 4 * ch + 4, :],
                    rb_sb[32 * ch : 32 * ch + 32, :],
                )
                for h in range(HPC):
                    bc = bcpool.tile([64, 512], BF16, tag="bc", name=f"bc_{ch}_{h}")
                    nc.sync.dma_start(
                        bc[:],
                        recip_d[4 * ch + h : 4 * ch + h + 1, :].to_broadcast(
                            (64, 512)
                        ),
                    )
                    if h % 2 == 0:
                        nc.vector.tensor_tensor(
                            attp_sb[0:64, h // 2, ch * 512 : (ch + 1) * 512],
                            pvs[0:64, h, :],
                            bc[:],
                            ALU.mult,
                        )
                    else:
                        nc.vector.tensor_tensor(
                            atth_sb[:, h // 2, :], pvs[0:64, h, :], bc[:], ALU.mult
                        )
                        nc.sync.dma_start(
                            attp_sb[64:128, h // 2, ch * 512 : (ch + 1) * 512],
                            atth_sb[:, h // 2, :],
                        )

            # ---- emit: lead-in pieces, then the fused q-block loop ----
            for p in make_pieces(0):
                p()
            dma_xt(1)

            for ch in range(NQB):
                nkt = 4 * ch + 4
                if ch + 2 < NQB:
                    dma_xt(ch + 2)
                # work hosted inside this qb's kt loop
                early = make_pieces(ch + 1) if ch + 1 < NQB else []
                late = make_outproj_pieces(ch - 1, nc.vector) if ch >= 1 else []
                work = {}
                if early:
                    k1 = max(1, nkt - 3)
                    for i, it in enumerate(early):
                        work.setdefault(i * k1 // len(early), []).append(it)
                if late:
                    lo = min(5, nkt - 3)
                    hi = min(nkt, lo + 4)
                    for i, it in enumerate(late):
                        work.setdefault(
                            lo + i * (hi - lo) // len(late), []
                        ).append(it)

                pvs = pspool.tile([65, HPC, QB], F32, tag="acc", name=f"pvs_{ch}")
                pv_stash = []
                for kt in range(nkt):
                    j = kt - 4 * ch  # >= 0 on diagonal tiles
                    qlo = max(0, j * 128)
                    g0 = ch * 512 + qlo
                    g1 = (ch + 1) * 512
                    for w in range(2):
                        psc = pspool.tile(
                            [128, 2, 512], F32, tag=f"psc{w}",
                            name=f"psc{w}_{ch}_{kt}",
                        )
                        for hh in range(2):
                            nc.tensor.matmul(
                                psc[:, hh, qlo:512],
                                km[w][
                                    64 * hh : 64 * hh + 64,
                                    kt * 128 : (kt + 1) * 128,
                                ],
                                qm[w][64 * hh : 64 * hh + 64, g0:g1],
                                start=True,
                                stop=True,
                                tile_position=(64 * hh, 0),
                            )
                        pt = ptpool.tile(
                            [128, 2, 512], BF16, tag="pt", name=f"pt{w}_{ch}_{kt}"
                        )
                        nc.scalar.activation(
                            pt[:, :, qlo:512], psc[:, :, qlo:512], AF.Exp, scale=0.125
                        )
                        if j >= 0:
                            nc.vector.tensor_tensor(
                                pt[:, :, qlo : qlo + 128],
                                pt[:, :, qlo : qlo + 128],
                                tri_b2,
                                ALU.mult,
                            )

                        def pv_emit(kt=kt, w=w, pt=pt, qlo=qlo):
                            for hh in range(2):
                                h = 2 * w + hh
                                nc.tensor.matmul(
                                    pvs[:, h, qlo:512],
                                    v_sb[:, kt, 65 * h : 65 * h + 65],
                                    pt[:, hh, qlo:512],
                                    start=(kt == 0),
                                    stop=(kt == nkt - 1),
                                )

                        pv_stash.append(pv_emit)
                    # deferred PV emission (DD kt's behind scores/exp)
                    while len(pv_stash) > 2 * DD:
                        pv_stash.pop(0)()
                    for it in work.get(kt, []):
                        it()
                for f in pv_stash:
                    f()
                pv_stash.clear()

                norm_block(ch, pvs)

            # last q-block's out-projection + RS (ACT is idle by now)
            for p in make_outproj_pieces(NQB - 1, nc.scalar):
                p()

    nc.compile()
    return nc


def make_in_maps(x, Wq, bq, Wk, bk, Wv, bv, Wo):
    inv = 1.0 / (ROPE_BASE ** (2.0 * np.arange(32, dtype=np.float64) / L))
    ang = np.arange(S, dtype=np.float64)[:, None] * inv[None, :]  # [S, 32]
    cos4 = np.tile(np.cos(ang).T, (HPC, 1)).astype(NPBF16)  # [128, S]
    sin4 = np.tile(np.sin(ang).T, (HPC, 1)).astype(NPBF16)
    tri = (np.arange(128)[None, :] >= np.arange(128)[:, None]).astype(NPBF16)

    in_maps = []
    for c in range(N_CORES):
        b, g = divmod(c, HPC)
        even = np.concatenate([64 * h + 2 * np.arange(32) for h in range(4 * g, 4 * g + 4)])
        odd = even + 1
        vcols = np.arange(256 * g, 256 * (g + 1))
        in_maps.append(
            {
                "xt": np.ascontiguousarray(x[b].T).astype(NPBF16),
                "wq0": np.ascontiguousarray(Wq[:, even]).astype(NPBF16),
                "wq1": np.ascontiguousarray(Wq[:, odd]).astype(NPBF16),
                "wk0": np.ascontiguousarray(Wk[:, even]).astype(NPBF16),
                "wk1": np.ascontiguousarray(Wk[:, odd]).astype(NPBF16),
                "wv": np.ascontiguousarray(Wv[:, vcols]).astype(NPBF16),
                "wo": np.ascontiguousarray(Wo[vcols, :]).astype(NPBF16),
                "bq0": bq[even].reshape(128, 1).astype(np.float32),
                "bq1": bq[odd].reshape(128, 1).astype(np.float32),
                "bk0": bk[even].reshape(128, 1).astype(np.float32),
                "bk1": bk[odd].reshape(128, 1).astype(np.float32),
                "bvr": bv[vcols].reshape(1, 256).astype(NPBF16),
                "cos4": cos4,
                "sin4": sin4,
                "tri": tri,
            }
        )
    return in_maps


def assemble_output(results, bo):
    out = np.empty((B, S, D), np.float32)
    for c in range(N_CORES):
        b, g = divmod(c, HPC)
        sh = np.asarray(results[c]["out"]).astype(np.float32).reshape(NQB, 128, D)
        for qb in range(NQB):
            r0 = qb * 512 + g * 128
            out[b, r0 : r0 + 128, :] = sh[qb]
    # bo is added once, after the reduction (matches `attended @ Wo + bo`).
    out += bo[None, None, :].astype(np.float32)
    return out


_CACHE = {}


def kernel(x, Wq, bq, Wk, bk, Wv, bv, Wo, bo, **run_kwargs):
    if "nc" not in _CACHE:
        _CACHE["nc"] = build_program()
    nc = _CACHE["nc"]
    in_maps = make_in_maps(
        np.asarray(x), np.asarray(Wq), np.asarray(bq), np.asarray(Wk),
        np.asarray(bk), np.asarray(Wv), np.asarray(bv), np.asarray(Wo),
    )
    res = bass_utils.run_bass_kernel_spmd(
        nc, in_maps, core_ids=list(range(N_CORES)), **run_kwargs
    )
    out = assemble_output(res.results, np.asarray(bo))
    kernel.last_results = res
    return out
